# revision 16
# baseline (speedup 1.0000x reference)
"""TRN2 Bass kernel for nn_Augment: homography bilinear warp + gamma +
color matrix + cutout, data-parallel over 8 NeuronCores.

v7 architecture (v4 with decongested band loads):
- 16w x 32h output tiles (512 px). Each Q7 core (16 partitions) handles 4
  tiles/round via 4 tap-partitions per tile. 32 tiles per NeuronCore round
  (one full 32-row strip of one image), 64 rounds.
- Band HBM loads: ONE copy per tile (dynamic-register offset, 16 tiles per
  HWDGE ring), halving the descriptor load on the two dynamic rings vs
  loading x-shifted pairs. The 3 shifted tap copies (x+1, y+1, both) are
  produced by 3 static SBUF->SBUF DMAs (in-partition shifts of 3 / CB3 /
  CB3+3 elems, partitions p0 -> p0+{1,2,3}).
- 16 offset registers per engine, all loaded back-to-back before the 16
  dma_starts, so reg_load latency pipelines instead of serializing
  load->use->load->use.
- Band issue for round r+1 is emitted right after round r's gather so the
  ring time overlaps the round's compute.
- Geometry computed densely in two wrap layouts [128,128] (no replication):
  layout A (gather index order) for idx, layout B (transpose order) for the
  4 bilinear tap weights (+valid +cutout folded in).
- Tap weights transported to gather-output alignment with a single DVE
  32x32 stream transpose.
- Bilinear combine via TensorE matmuls into PSUM (4 accumulation phases
  select each tile's 4 tap partitions), then gamma (ScalarE) + color
  matrix (DVE, fused scalar_tensor_tensor) + clips.

Self-contained: hardcodes shapes (B=32, H=W=512, C=3, 8 cores).
"""
import os
import sys
import types
import numpy as np

import concourse.bass as bass
import concourse.bacc as bacc
import concourse.mybir as mybir
import concourse.tile as tile
from concourse.bass_utils import run_bass_kernel_spmd
from concourse.tile_rust import add_dep_helper

B, H, W, CH = 32, 512, 512, 3
NCORES = 8
IPC = B // NCORES
P = 128
F32 = mybir.dt.float32
I32 = mybir.dt.int32
I16 = mybir.dt.int16

TILE_H, TILE_W = 32, 16
NTY, NTX = H // TILE_H, W // TILE_W          # 16 x 32
TPI = NTY * NTX                              # 512 tiles/image
TPR = 32                                     # tiles per round (one strip)
NROUND_FULL = IPC * TPI // TPR               # 64
S = TILE_H * TILE_W                          # 512 px/tile
POS = 4 * S                                  # 2048 positions per Q7 core
ROWB = W * CH
IMGEL = H * ROWB
SCL = 2.0 / 511.0

# ---- coef column layout ----
# layout A (idx): slopes (per-partition scalars) + per-quarter intercepts
A_SU, A_SV, A_SD = 0, 1, 2
A_BU, A_BV, A_BD, A_IDXB = 3, 7, 11, 15     # 4 cols each
# layout B (weights): slopes + per-(h,q) intercepts (8 segs)
B_SU, B_SV, B_SD = 19, 20, 21
B_BU, B_BV, B_BD, B_CLY, B_CHY = 22, 30, 38, 46, 54   # 8 cols each
# dense layout: gamma + color matrix
D_G = 62
D_M = 63                                     # 9 cols
NCOEF = 72


def _install_profhook():
    if "antenv.axon_hooks" in sys.modules:
        return
    box = [None]
    m = types.ModuleType("antenv.axon_hooks")
    m.set_axon_ntff_profile_hook = lambda h: box.__setitem__(0, h)
    m.get_axon_ntff_profile_hook = lambda: box[0]
    sys.modules["antenv.axon_hooks"] = m
    try:
        import antenv
        antenv.axon_hooks = m
        from trn_agent_boot.trn_boot import _ntff_profile_via_ctypes
        box[0] = _ntff_profile_via_ctypes("/opt/axon/libaxon_pjrt.so")
    except Exception:
        pass


def host_params(geom_u, color_u, cutout_u):
    g = geom_u.astype(np.float64)
    flip = geom_u[:, 0] > 0.5
    tilt = (g[:, 1] * 2 - 1) * (15 * np.pi / 180)
    pan = (g[:, 2] * 2 - 1) * (15 * np.pi / 180)
    rot = (g[:, 3] * 2 - 1) * (15 * np.pi / 180)
    sc = 1.0 + (g[:, 4] * 2 - 1) * 0.1
    tx = (g[:, 5] * 2 - 1) * 0.2
    ty = (g[:, 6] * 2 - 1) * 0.2
    n = geom_u.shape[0]
    Ms = np.zeros((n, 3, 3), np.float64)
    for i in range(n):
        f = -1.0 if flip[i] else 1.0
        F = np.diag([f, 1.0, 1.0])
        ct, st = np.cos(tilt[i]), np.sin(tilt[i])
        Rx = np.array([[1, 0, 0], [0, ct, -st], [0, st, ct]])
        cp, sp = np.cos(pan[i]), np.sin(pan[i])
        Ry = np.array([[cp, 0, sp], [0, 1, 0], [-sp, 0, cp]])
        cr, sr = np.cos(rot[i]), np.sin(rot[i])
        RS = np.array([[sc[i] * cr, -sc[i] * sr, 0],
                       [sc[i] * sr, sc[i] * cr, 0], [0, 0, 1]])
        T = np.array([[1, 0, tx[i]], [0, 1, ty[i]], [0, 0, 1]])
        Ms[i] = T @ RS @ Rx @ Ry @ F
    cu = color_u.astype(np.float64)
    gamma = 1.0 + (cu[:, 3] * 2 - 1) * 0.2
    theta = (cu[:, 0] * 2 - 1) * (10 * np.pi / 180)
    c, s = np.cos(theta), np.sin(theta)
    a = 1.0 / np.sqrt(3.0)
    I3 = np.eye(3)
    K = np.array([[0, -a, a], [a, 0, -a], [-a, a, 0]])
    A = np.full((3, 3), 1.0 / 3.0)
    sat = 1.0 + (cu[:, 1] * 2 - 1) * 0.4
    lum = np.array([0.299, 0.587, 0.114])
    L = np.broadcast_to(lum, (3, 3))
    bright = 1.0 + (cu[:, 2] * 2 - 1) * 0.1
    Mc = np.zeros((n, 3, 3), np.float64)
    for i in range(n):
        Rh = c[i] * I3 + s[i] * K + (1 - c[i]) * A
        Sm = sat[i] * I3 + (1 - sat[i]) * L
        Mc[i] = bright[i] * (Sm @ Rh)
    qu = cutout_u.astype(np.float64)
    capply = qu[:, 0] < 0.5
    return (Ms, Mc, gamma, capply, qu[:, 1], qu[:, 2],
            0.3 + 0.2 * qu[:, 3], 0.3 + 0.2 * qu[:, 4])


def tile_bbox(M, ty, txi):
    ys = np.array([ty * TILE_H, ty * TILE_H + TILE_H - 1], np.float64)
    xs = np.array([txi * TILE_W, txi * TILE_W + TILE_W - 1], np.float64)
    Xg, Yg = np.meshgrid(xs, ys)
    xn = Xg * SCL - 1
    yn = Yg * SCL - 1
    nu = M[0, 0] * xn + M[0, 1] * yn + M[0, 2]
    nv = M[1, 0] * xn + M[1, 1] * yn + M[1, 2]
    de = M[2, 0] * xn + M[2, 1] * yn + M[2, 2]
    px = np.clip((nu / de + 1) * 0.5 * 511, 0, 511)
    py = np.clip((nv / de + 1) * 0.5 * 511, 0, 511)
    return (int(np.floor(py.min())), int(np.floor(py.max())),
            int(np.floor(px.min())), int(np.floor(px.max())))


def global_band_shape(Ms):
    RB = CB = 0
    for bi in range(Ms.shape[0]):
        for ty in range(NTY):
            for txi in range(NTX):
                y0, y1, x0, x1 = tile_bbox(Ms[bi], ty, txi)
                RB = max(RB, y1 - y0 + 3)
                CB = max(CB, x1 - x0 + 3)
    assert (RB + 1) * CB * CH <= 2**15, f"band too big: {RB}x{CB}"
    return RB, CB


def build_tables(Ms, Mc, gamma, capply, ccx, ccy, ccw, cch, core, nround,
                 RB, CB, sim_floor):
    dmaoff = np.zeros((nround, TPR), np.int32)
    coef = np.zeros((nround, P, NCOEF), np.float64)
    pidx = np.arange(P)
    # A floor fold: on HW cast rounds-to-nearest -> px-0.5; sim cast truncs.
    FA = 255.5 if sim_floor else 255.0
    for r in range(nround):
        imgl, strip = r // 16, r % 16
        bi = core * IPC + imgl
        M = Ms[bi]
        TY0 = 32 * strip
        t0 = SCL * TY0 - 1.0
        cf = coef[r]
        # slopes (same for every partition; image constant per round)
        cf[:, A_SU] = (255.5 * M[0, 1] + FA * M[2, 1]) * SCL
        cf[:, A_SV] = (255.5 * M[1, 1] + FA * M[2, 1]) * SCL
        cf[:, A_SD] = M[2, 1] * SCL
        cf[:, B_SU] = 2 * (255.5 * M[0, 1] + 255.5 * M[2, 1]) * SCL
        cf[:, B_SV] = 2 * (255.5 * M[1, 1] + 255.5 * M[2, 1]) * SCL
        cf[:, B_SD] = 2 * M[2, 1] * SCL
        # cutout box (image coords)
        if capply[bi]:
            lx = 511.0 * (ccx[bi] - ccw[bi] * 0.5)
            hx = 511.0 * (ccx[bi] + ccw[bi] * 0.5)
            ly = 511.0 * (ccy[bi] - cch[bi] * 0.5)
            hy = 511.0 * (ccy[bi] + cch[bi] * 0.5)
        # per-tile data
        Y0s = np.zeros(TPR, np.int64)
        X0s = np.zeros(TPR, np.int64)
        for t in range(TPR):
            y0, y1, x0, x1 = tile_bbox(M, strip, t)
            Y0 = max(0, min(y0 - 1, H - 1 - RB))
            X0 = max(0, min(x0 - 1, W - 1 - CB))
            Y0s[t], X0s[t] = Y0, X0
            dmaoff[r, t] = imgl * IMGEL + Y0 * ROWB + X0 * CH
        # ---- layout A (g = p//16, m = p%16, quarter q -> tile 4g+q) ----
        for q in range(4):
            tt = (pidx // 16) * 4 + q           # tile per partition
            TX0 = 16 * tt
            xn = SCL * (TX0 + (pidx % 16)) - 1.0
            nui = M[0, 0] * xn + M[0, 1] * t0 + M[0, 2]
            nvi = M[1, 0] * xn + M[1, 1] * t0 + M[1, 2]
            dei = M[2, 0] * xn + M[2, 1] * t0 + M[2, 2]
            cf[:, A_BU + q] = 255.5 * nui + FA * dei
            cf[:, A_BV + q] = 255.5 * nvi + FA * dei
            cf[:, A_BD + q] = dei
            cf[:, A_IDXB + q] = -(Y0s[tt] * (CB + 1) + X0s[tt])
        # ---- layout B (G = p//32, kk = p%32, segs (h,q)) ----
        kk = pidx % 32
        xB = kk % 16
        par = kk // 16
        ypart = SCL * (TY0 + par) - 1.0
        for h in range(2):
            for q in range(4):
                tt = ((pidx // 32) * 2 + h) * 4 + q
                TX0 = 16 * tt
                xn = SCL * (TX0 + xB) - 1.0
                nui = M[0, 0] * xn + M[0, 1] * ypart + M[0, 2]
                nvi = M[1, 0] * xn + M[1, 1] * ypart + M[1, 2]
                dei = M[2, 0] * xn + M[2, 1] * ypart + M[2, 2]
                sg = 4 * h + q
                cf[:, B_BU + sg] = 255.5 * (nui + dei)
                cf[:, B_BV + sg] = 255.5 * (nvi + dei)
                cf[:, B_BD + sg] = dei
                if capply[bi]:
                    xg = TX0 + xB
                    xin = (xg >= np.floor(lx) + 1) & (xg <= np.ceil(hx) - 1)
                    cly = np.where(xin, np.floor(ly) + 0.5 - TY0, 5000.0)
                    chy = np.where(xin, np.ceil(hy) - 0.5 - TY0, -5000.0)
                else:
                    cly, chy = 5000.0, -5000.0
                cf[:, B_CLY + sg] = cly
                cf[:, B_CHY + sg] = chy
        # ---- dense (tile = p//4) ----
        cf[:, D_G] = gamma[bi]
        cf[:, D_M:D_M + 9] = Mc[bi].reshape(-1)
    return dmaoff, coef.astype(np.float32)


def build_wsel():
    # phase (b, q): out partition 32*b + tile (tile = 4g+q) sums the 4 tap
    # partitions 16g + 4q + t; moving slice is gout[:, 1536q + 384b : +384].
    ws = np.zeros((16, P, P), np.float32)
    for bq in range(16):
        b, q = bq // 4, bq % 4
        for g in range(8):
            i = 32 * b + 4 * g + q
            for t in range(4):
                ws[bq, 16 * g + 4 * q + t, i] = 1.0
    return ws.transpose(1, 0, 2).reshape(P, 16 * P).copy()


def build_kernel(RB, CB, nround, sim_floor=False):
    nc = bacc.Bacc("TRN2", target_bir_lowering=False, debug=False,
                   enable_asserts=False, num_devices=NCORES)
    img = nc.dram_tensor("img", [IPC * IMGEL], F32, kind="ExternalInput")
    dmaoff_d = nc.dram_tensor("dmaoff", [nround, TPR], I32,
                              kind="ExternalInput")
    coef_d = nc.dram_tensor("coef", [nround, P, NCOEF], F32,
                            kind="ExternalInput")
    wsel_d = nc.dram_tensor("wsel", [P, 16 * P], F32, kind="ExternalInput")
    pconst_d = nc.dram_tensor("pconst", [P, 1], F32, kind="ExternalInput")
    out_d = nc.dram_tensor("out", [IPC * IMGEL], F32, kind="ExternalOutput")
    CBL = CB + 1                          # band row width: +1 px so the
    CB3 = CBL * CH                        # x+1 shift stays within the row
    BSZ = (RB + 1) * CB3                  # elems per band copy (p0)
    GSZ = RB * CB3                        # elems the gather can address
    AX = mybir.AluOpType
    AF = mybir.ActivationFunctionType
    FLOFF = 0.0 if sim_floor else 0.5
    NREG = 8                              # offset registers per engine

    with tile.TileContext(nc) as tc:
        with tc.tile_pool(name="cpool", bufs=1) as cpool, \
             tc.tile_pool(name="wpool", bufs=2) as wpool, \
             tc.tile_pool(name="ppool", bufs=2, space="PSUM") as ppool:
            # ---- persistent constants ----
            ii = cpool.tile([P, P], I32, name="ii")
            yA = cpool.tile([P, P], F32, name="yA")
            uB = cpool.tile([P, P], F32, name="uB")
            yinB = cpool.tile([P, P], F32, name="yinB")
            parc = cpool.tile([P, 1], F32, name="parc")
            wsel = cpool.tile([P, 16 * P], F32, name="wsel")
            nc.sync.dma_start(wsel[:], wsel_d[:, :])
            # yA: value = k % 32 over cols k
            nc.gpsimd.iota(ii[:], [[0, 4], [1, 32]], channel_multiplier=0)
            nc.vector.tensor_copy(yA[:], ii[:])
            # uB: cols (h:2, q:4, u16:16) -> u16
            nc.gpsimd.iota(ii[:], [[0, 2], [0, 4], [1, 16]],
                           channel_multiplier=0)
            nc.vector.tensor_copy(uB[:], ii[:])
            # par = (p % 32) >= 16 per partition (host-provided)
            nc.sync.dma_start(parc[:], pconst_d[:, :])
            # yinB = 2*u16 + par
            nc.vector.tensor_scalar(yinB[:], uB[:], 2.0, None, AX.mult)
            nc.vector.tensor_scalar(yinB[:], yinB[:], parc[:, 0:1], None,
                                    AX.add)

            # ---- persistent w_in (unused cols stay zero/stale-finite) ----
            w_in = cpool.tile([P, POS], F32, name="w_in")
            nc.gpsimd.memset(w_in[:], 0.0)

            # ---- manually double-buffered band tiles ----
            bands = [cpool.tile([P, BSZ], F32, name=f"band{i}")
                     for i in range(2)]
            # p1/p2/p3 tap copies fall a few elems short of the full band
            # extent; zero both buffers once so those cells stay finite.
            for bd in bands:
                nc.gpsimd.memset(bd[:], 0.0)

            # offset registers: NREG per HWDGE engine so reg_loads pipeline
            regs = {"sync": [], "scalar": []}
            for nm, et in (("sync", mybir.EngineType.SP),
                           ("scalar", mybir.EngineType.Activation)):
                for i in range(NREG):
                    regs[nm].append(nc.alloc_register(et, f"boff_{nm}{i}"))

            bos = [cpool.tile([1, TPR], I32, name=f"bo{i}")
                   for i in range(2)]
            cfs = [cpool.tile([P, NCOEF], F32, name=f"cf{i}")
                   for i in range(3)]

            def load_cf(r):
                nc.scalar.dma_start(
                    cfs[r % 3][:],
                    coef_d[r:r + 1, :, :].rearrange("o p c -> (o p) c"))

            def issue_loads(r):
                bd, bo = bands[r % 2], bos[r % 2]
                i_bo = nc.sync.dma_start(bo[:], dmaoff_d[r:r + 1, :])
                # batch reg_loads, then the dma_starts that consume them
                for half, eng in ((0, "sync"), (1, "scalar")):
                    e = getattr(nc, eng)
                    for blk in range(16 // NREG):
                        for i in range(NREG):
                            t = 16 * half + blk * NREG + i
                            rl = nc.reg_load(regs[eng][i], bo[0:1, t:t + 1])
                            add_dep_helper(rl.ins, i_bo.ins, sync=True,
                                           reason="boff")
                        for i in range(NREG):
                            t = 16 * half + blk * NREG + i
                            src = bass.AP(img, regs[eng][i],
                                          [[ROWB, RB + 1], [1, CB3]])
                            p0 = 4 * t
                            e.dma_start(bd[p0:p0 + 1, 0:BSZ], src)

            def issue_reps(r):
                # tap-shift replication: p0 -> p1 (x+1), p2 (y+1), p3 (both)
                bd = bands[r % 2]
                if sim_floor:
                    # interp can't handle strided-partition SBUF copies
                    for t in range(TPR):
                        p0 = 4 * t
                        nc.sync.dma_start(bd[p0 + 1:p0 + 2, 0:GSZ],
                                          bd[p0:p0 + 1, 3:GSZ + 3])
                        nc.scalar.dma_start(bd[p0 + 2:p0 + 3, 0:GSZ],
                                            bd[p0:p0 + 1, CB3:CB3 + GSZ])
                        nc.sync.dma_start(bd[p0 + 3:p0 + 4, 0:GSZ - 3],
                                          bd[p0:p0 + 1, CB3 + 3:CB3 + GSZ])
                else:
                    nc.sync.dma_start(bd[1:P:4, 0:GSZ],
                                      bd[0:P:4, 3:GSZ + 3])
                    nc.scalar.dma_start(bd[2:P:4, 0:GSZ],
                                        bd[0:P:4, CB3:CB3 + GSZ])
                    nc.sync.dma_start(bd[3:P:4, 0:GSZ - 3],
                                      bd[0:P:4, CB3 + 3:CB3 + GSZ])

            def emit_layouts(r):
                """DVE block computing round r's gather indices and tap
                weights; in steady state it runs during round r-1."""
                cf = cfs[r % 3]

                def sc(i):
                    return cf[:, i:i + 1]

                def bcA(i):
                    b = cf[:, i:i + 4]
                    return bass.AP(b.tensor, b.offset,
                                   [b.ap[0], [1, 4], [0, 32]])

                def bcB(i):
                    b = cf[:, i:i + 8]
                    return bass.AP(b.tensor, b.offset,
                                   [b.ap[0], [1, 8], [0, 16]])

                # ---- layout A: gather indices ----
                de = wpool.tile([P, P], F32, tag="de")
                t1 = wpool.tile([P, P], F32, tag="t1")
                t2 = wpool.tile([P, P], F32, tag="t2")
                x0f = wpool.tile([P, P], F32, tag="x0f")
                y0f = wpool.tile([P, P], F32, tag="y0f")
                s_i = wpool.tile([P, P], I32, tag="s_i")
                idx16 = wpool.tile([P, P], I16, tag="idx16")
                nc.vector.scalar_tensor_tensor(
                    de[:], yA[:], sc(A_SD), bcA(A_BD), AX.mult, AX.add)
                nc.vector.reciprocal(de[:], de[:])
                for dst, scol, bcol in ((x0f, A_SU, A_BU), (y0f, A_SV, A_BV)):
                    nc.vector.scalar_tensor_tensor(
                        t1[:], yA[:], sc(scol), bcA(bcol), AX.mult, AX.add)
                    nc.vector.tensor_tensor(t1[:], t1[:], de[:], AX.mult)
                    nc.vector.tensor_copy(s_i[:], t1[:])
                    nc.vector.tensor_copy(dst[:], s_i[:])
                    nc.vector.tensor_scalar(dst[:], dst[:], 0.0, 510.0,
                                            AX.max, AX.min)
                # idx = y0f*CBL + x0f + bias -> i16
                nc.vector.tensor_scalar(y0f[:], y0f[:], float(CBL), None,
                                        AX.mult)
                nc.vector.tensor_tensor(y0f[:], y0f[:], x0f[:], AX.add)
                nc.vector.tensor_tensor(y0f[:], y0f[:], bcA(A_IDXB), AX.add)
                nc.vector.tensor_copy(s_i[:], y0f[:])
                nc.vector.tensor_copy(idx16[:], s_i[:])

                # ---- layout B: tap weights ----
                pxu = wpool.tile([P, P], F32, tag="pxu")
                pyu = wpool.tile([P, P], F32, tag="pyu")
                pxc = wpool.tile([P, P], F32, tag="pxc")
                pyc = wpool.tile([P, P], F32, tag="pyc")
                vv = wpool.tile([P, P], F32, tag="vv")
                nc.vector.scalar_tensor_tensor(
                    de[:], uB[:], sc(B_SD), bcB(B_BD), AX.mult, AX.add)
                nc.vector.reciprocal(de[:], de[:])
                for dst, dstc, scol, bcol in ((pxu, pxc, B_SU, B_BU),
                                              (pyu, pyc, B_SV, B_BV)):
                    nc.vector.scalar_tensor_tensor(
                        t1[:], uB[:], sc(scol), bcB(bcol), AX.mult, AX.add)
                    nc.vector.tensor_tensor(dst[:], t1[:], de[:], AX.mult)
                    nc.vector.tensor_scalar(dstc[:], dst[:], 0.0, 511.0,
                                            AX.max, AX.min)
                # valid
                nc.vector.tensor_tensor(pxu[:], pxu[:], pxc[:], AX.is_equal)
                nc.vector.tensor_tensor(pyu[:], pyu[:], pyc[:], AX.is_equal)
                nc.vector.tensor_tensor(vv[:], pxu[:], pyu[:], AX.mult)
                # cutout: ym in (CLY, CHY) -> zero weight
                nc.vector.tensor_tensor(t1[:], yinB[:], bcB(B_CLY), AX.is_ge)
                nc.vector.tensor_tensor(t2[:], yinB[:], bcB(B_CHY), AX.is_le)
                nc.vector.tensor_tensor(t1[:], t1[:], t2[:], AX.mult)
                nc.vector.tensor_scalar(t1[:], t1[:], -1.0, 1.0,
                                        AX.mult, AX.add)
                nc.vector.tensor_tensor(vv[:], vv[:], t1[:], AX.mult)
                # floors + fracs
                for pc, fl in ((pxc, x0f), (pyc, y0f)):
                    nc.vector.tensor_scalar(t1[:], pc[:], 1.0, -FLOFF,
                                            AX.mult, AX.add)
                    nc.vector.tensor_copy(s_i[:], t1[:])
                    nc.vector.tensor_copy(fl[:], s_i[:])
                    nc.vector.tensor_scalar(fl[:], fl[:], 0.0, 510.0,
                                            AX.max, AX.min)
                    nc.vector.tensor_tensor(pc[:], pc[:], fl[:], AX.subtract)
                # pxc=wx, pyc=wy now. basis: t1=A1=V*wx, t2=A0=V-A1, vv ->
                nc.vector.tensor_tensor(t1[:], vv[:], pxc[:], AX.mult)
                nc.vector.tensor_tensor(t2[:], vv[:], t1[:], AX.subtract)
                # B0 = 1-wy -> pxu (reuse)
                nc.vector.tensor_scalar(pxu[:], pyc[:], -1.0, 1.0,
                                        AX.mult, AX.add)
                # tap writes into w_in (cols 516q + 32u16 + 16h + t)
                for hh in range(2):
                    for t in range(4):
                        adx = t1 if (t & 1) else t2
                        bdy = pyc if (t & 2) else pxu
                        sb = adx[:, 64 * hh:64 * hh + 64]
                        sv = bass.AP(sb.tensor, sb.offset,
                                     [sb.ap[0], [16, 4], [1, 16]])
                        db = w_in[:, 16 * hh + t:]
                        dv = bass.AP(db.tensor, db.offset,
                                     [db.ap[0], [516, 4], [32, 16]])
                        bb = bdy[:, 64 * hh:64 * hh + 64]
                        bv = bass.AP(bb.tensor, bb.offset,
                                     [bb.ap[0], [16, 4], [1, 16]])
                        nc.vector.tensor_tensor(dv, sv, bv, AX.mult)
                # transpose to gather alignment
                w_out = wpool.tile([P, POS], F32, tag="w_out")
                nc.vector.transpose(w_out[:], w_in[:])
                return idx16, w_out

            # ---- prologue: prime round 0 ----
            load_cf(0)
            issue_loads(0)
            issue_reps(0)
            if nround > 1:
                load_cf(1)
            idx_cur, w_cur = emit_layouts(0)

            for r in range(nround):
                band = bands[r % 2]
                cf = cfs[r % 3]

                def scr(i, cf=cf):
                    return cf[:, i:i + 1]

                if r + 2 < nround:
                    load_cf(r + 2)

                # ---- gather (indices computed during round r-1) ----
                gout = wpool.tile([P, POS * CH], F32, tag="gout")
                i_gth = nc.gpsimd.ap_gather(
                    gout[:].rearrange("p (n d) -> p n d", d=CH),
                    band[:, 0:GSZ].rearrange("p (n d) -> p n d", d=CH),
                    idx_cur[:], channels=P, num_elems=RB * CBL, d=CH,
                    num_idxs=POS)

                # next round's band + DVE layouts overlap this round
                if r + 1 < nround:
                    issue_loads(r + 1)
                    idx_nxt, w_nxt = emit_layouts(r + 1)
                else:
                    idx_nxt = w_nxt = None

                # ---- apply weights (computed during round r-1) ----
                wb = bass.AP(w_cur.tensor, w_cur[:].offset,
                             [w_cur[:].ap[0], [1, POS], [0, CH]])
                i_wm = nc.gpsimd.tensor_tensor(
                    gout[:].rearrange("p (n d) -> p n d", d=CH),
                    gout[:].rearrange("p (n d) -> p n d", d=CH),
                    wb, AX.mult)
                add_dep_helper(i_wm.ins, i_gth.ins, sync=True, reason="graw")

                if r + 1 < nround:
                    issue_reps(r + 1)

                # ---- combine via matmul: psum IS the dense layout ----
                pt = ppool.tile([P, 3 * S // 4], F32, tag="psum")
                for bq in range(16):
                    b, q = bq // 4, bq % 4
                    nc.tensor.matmul(
                        pt[:, :],
                        wsel[:, P * bq:P * (bq + 1)],
                        gout[:, 1536 * q + 384 * b:
                             1536 * q + 384 * b + 384],
                        start=(bq == 0), stop=(bq == 15))

                # ---- post: clip, gamma, color, clip ----
                d_t = wpool.tile([P, 3 * S // 4], F32, tag="d_t")
                nc.vector.tensor_scalar(pt[:], pt[:], 1e-30, 1.0,
                                        AX.max, AX.min)
                nc.scalar.activation(d_t[:], pt[:], AF.Ln)
                nc.scalar.activation(d_t[:], d_t[:], AF.Exp,
                                     scale=cf[:, D_G:D_G + 1])
                d_c = wpool.tile([P, 3 * S // 4], F32, tag="d_c")
                dv3 = d_t[:].rearrange("p (n c) -> p n c", c=CH)
                tv3 = d_c[:].rearrange("p (n c) -> p n c", c=CH)
                for c3 in range(3):
                    nc.vector.tensor_scalar(
                        tv3[:, :, c3:c3 + 1], dv3[:, :, 0:1],
                        scr(D_M + 3 * c3), None, AX.mult)
                    nc.vector.scalar_tensor_tensor(
                        tv3[:, :, c3:c3 + 1], dv3[:, :, 1:2],
                        scr(D_M + 3 * c3 + 1), tv3[:, :, c3:c3 + 1],
                        AX.mult, AX.add)
                    nc.vector.scalar_tensor_tensor(
                        tv3[:, :, c3:c3 + 1], dv3[:, :, 2:3],
                        scr(D_M + 3 * c3 + 2), tv3[:, :, c3:c3 + 1],
                        AX.mult, AX.add)
                nc.vector.tensor_scalar(d_c[:], d_c[:], 0.0, 1.0,
                                        AX.max, AX.min)

                # ---- output stores ----
                imgl, strip = r // 16, r % 16
                for r2 in range(8):
                    off = imgl * IMGEL + (32 * strip + r2) * ROWB
                    dst = bass.AP(out_d, off,
                                  [[8 * ROWB, 4], [48, 32], [1, 48]])
                    e = nc.sync if r2 % 2 == 0 else nc.scalar
                    e.dma_start(dst, d_c[:, 48 * r2:48 * r2 + 48])

                idx_cur, w_cur = idx_nxt, w_nxt
    nc.compile()
    return nc


_CACHE = {}


def kernel(images, geom_u, color_u, cutout_u):
    _install_profhook()
    images = np.asarray(images, dtype=np.float32)
    geom_u = np.asarray(geom_u, dtype=np.float32)
    color_u = np.asarray(color_u, dtype=np.float32)
    cutout_u = np.asarray(cutout_u, dtype=np.float32)
    Ms, Mc, gamma, capply, ccx, ccy, ccw, cch = host_params(
        geom_u, color_u, cutout_u)
    RB, CB = global_band_shape(Ms)
    nround = int(os.environ.get('KROUNDS', NROUND_FULL))
    sim_floor = os.environ.get("KSIM", "0") == "1"
    key = (RB, CB, nround, sim_floor)
    if key not in _CACHE:
        _CACHE[key] = build_kernel(RB, CB, nround, sim_floor=sim_floor)
    nc = _CACHE[key]
    wsel = build_wsel()
    in_maps = []
    for core in range(NCORES):
        dmaoff, coef = build_tables(Ms, Mc, gamma, capply, ccx, ccy, ccw,
                                    cch, core, nround, RB, CB, sim_floor)
        shard = images[core * IPC:(core + 1) * IPC].reshape(-1)
        pconst = (((np.arange(P) % 32) >= 16)
                  .astype(np.float32).reshape(P, 1))
        in_maps.append({"img": np.ascontiguousarray(shard),
                        "dmaoff": dmaoff, "coef": coef, "wsel": wsel,
                        "pconst": pconst})
    if sim_floor:
        from concourse.bass_interp import CoreSim
        sim = CoreSim(nc, trace=False)
        for k, v in in_maps[0].items():
            sim.tensor(k)[:] = v
        sim.simulate()
        out = np.zeros((B, H, W, CH), np.float32)
        out[0:IPC] = np.array(sim.tensor("out")).reshape(IPC, H, W, CH)
        return out
    trace = os.environ.get("KTRACE", "0") == "1"
    res = run_bass_kernel_spmd(nc, in_maps, core_ids=list(range(NCORES)),
                               trace=trace, trace_cores=[0] if trace else None)
    if trace and res.exec_time_ns:
        print(f"HW exec time: {res.exec_time_ns} ns")
    out = np.zeros((B, H, W, CH), np.float32)
    for core in range(NCORES):
        out[core * IPC:(core + 1) * IPC] = (
            res.results[core]["out"].reshape(IPC, H, W, CH))
    return out


# revision 17
# speedup vs baseline: 1.1938x; 1.1938x over previous
"""TRN2 Bass kernel for nn_Augment: homography bilinear warp + gamma +
color matrix + cutout, data-parallel over 8 NeuronCores.

v7 architecture (v4 with decongested band loads):
- 16w x 32h output tiles (512 px). Each Q7 core (16 partitions) handles 4
  tiles/round via 4 tap-partitions per tile. 32 tiles per NeuronCore round
  (one full 32-row strip of one image), 64 rounds.
- Band HBM loads: ONE copy per tile (dynamic-register offset, 16 tiles per
  HWDGE ring), halving the descriptor load on the two dynamic rings vs
  loading x-shifted pairs. The 3 shifted tap copies (x+1, y+1, both) are
  produced by 3 static SBUF->SBUF DMAs (in-partition shifts of 3 / CB3 /
  CB3+3 elems, partitions p0 -> p0+{1,2,3}).
- 16 offset registers per engine, all loaded back-to-back before the 16
  dma_starts, so reg_load latency pipelines instead of serializing
  load->use->load->use.
- Band issue for round r+1 is emitted right after round r's gather so the
  ring time overlaps the round's compute.
- Geometry computed densely in two wrap layouts [128,128] (no replication):
  layout A (gather index order) for idx, layout B (transpose order) for the
  4 bilinear tap weights (+valid +cutout folded in).
- Tap weights transported to gather-output alignment with a single DVE
  32x32 stream transpose.
- Bilinear combine via TensorE matmuls into PSUM (4 accumulation phases
  select each tile's 4 tap partitions), then gamma (ScalarE) + color
  matrix (DVE, fused scalar_tensor_tensor) + clips.

Self-contained: hardcodes shapes (B=32, H=W=512, C=3, 8 cores).
"""
import os
import sys
import types
import numpy as np

import concourse.bass as bass
import concourse.bacc as bacc
import concourse.mybir as mybir
import concourse.tile as tile
from concourse.bass_utils import run_bass_kernel_spmd
from concourse.tile_rust import add_dep_helper

B, H, W, CH = 32, 512, 512, 3
NCORES = 8
IPC = B // NCORES
P = 128
F32 = mybir.dt.float32
I32 = mybir.dt.int32
I16 = mybir.dt.int16

TILE_H, TILE_W = 32, 16
NTY, NTX = H // TILE_H, W // TILE_W          # 16 x 32
TPI = NTY * NTX                              # 512 tiles/image
TPR = 32                                     # tiles per round (one strip)
NROUND_FULL = IPC * TPI // TPR               # 64
S = TILE_H * TILE_W                          # 512 px/tile
POS = 4 * S                                  # 2048 positions per Q7 core
ROWB = W * CH
IMGEL = H * ROWB
SCL = 2.0 / 511.0

# ---- coef column layout ----
# layout A (idx): slopes (per-partition scalars) + per-quarter intercepts
A_SU, A_SV, A_SD = 0, 1, 2
A_BU, A_BV, A_BD, A_IDXB = 3, 7, 11, 15     # 4 cols each
# layout B (weights): slopes + per-(h,q) intercepts (8 segs)
B_SU, B_SV, B_SD = 19, 20, 21
B_BU, B_BV, B_BD, B_CLY, B_CHY = 22, 30, 38, 46, 54   # 8 cols each
# dense layout: gamma + color matrix
D_G = 62
D_M = 63                                     # 9 cols
NCOEF = 72


def _install_profhook():
    if "antenv.axon_hooks" in sys.modules:
        return
    box = [None]
    m = types.ModuleType("antenv.axon_hooks")
    m.set_axon_ntff_profile_hook = lambda h: box.__setitem__(0, h)
    m.get_axon_ntff_profile_hook = lambda: box[0]
    sys.modules["antenv.axon_hooks"] = m
    try:
        import antenv
        antenv.axon_hooks = m
        from trn_agent_boot.trn_boot import _ntff_profile_via_ctypes
        box[0] = _ntff_profile_via_ctypes("/opt/axon/libaxon_pjrt.so")
    except Exception:
        pass


def host_params(geom_u, color_u, cutout_u):
    g = geom_u.astype(np.float64)
    flip = geom_u[:, 0] > 0.5
    tilt = (g[:, 1] * 2 - 1) * (15 * np.pi / 180)
    pan = (g[:, 2] * 2 - 1) * (15 * np.pi / 180)
    rot = (g[:, 3] * 2 - 1) * (15 * np.pi / 180)
    sc = 1.0 + (g[:, 4] * 2 - 1) * 0.1
    tx = (g[:, 5] * 2 - 1) * 0.2
    ty = (g[:, 6] * 2 - 1) * 0.2
    n = geom_u.shape[0]
    Ms = np.zeros((n, 3, 3), np.float64)
    for i in range(n):
        f = -1.0 if flip[i] else 1.0
        F = np.diag([f, 1.0, 1.0])
        ct, st = np.cos(tilt[i]), np.sin(tilt[i])
        Rx = np.array([[1, 0, 0], [0, ct, -st], [0, st, ct]])
        cp, sp = np.cos(pan[i]), np.sin(pan[i])
        Ry = np.array([[cp, 0, sp], [0, 1, 0], [-sp, 0, cp]])
        cr, sr = np.cos(rot[i]), np.sin(rot[i])
        RS = np.array([[sc[i] * cr, -sc[i] * sr, 0],
                       [sc[i] * sr, sc[i] * cr, 0], [0, 0, 1]])
        T = np.array([[1, 0, tx[i]], [0, 1, ty[i]], [0, 0, 1]])
        Ms[i] = T @ RS @ Rx @ Ry @ F
    cu = color_u.astype(np.float64)
    gamma = 1.0 + (cu[:, 3] * 2 - 1) * 0.2
    theta = (cu[:, 0] * 2 - 1) * (10 * np.pi / 180)
    c, s = np.cos(theta), np.sin(theta)
    a = 1.0 / np.sqrt(3.0)
    I3 = np.eye(3)
    K = np.array([[0, -a, a], [a, 0, -a], [-a, a, 0]])
    A = np.full((3, 3), 1.0 / 3.0)
    sat = 1.0 + (cu[:, 1] * 2 - 1) * 0.4
    lum = np.array([0.299, 0.587, 0.114])
    L = np.broadcast_to(lum, (3, 3))
    bright = 1.0 + (cu[:, 2] * 2 - 1) * 0.1
    Mc = np.zeros((n, 3, 3), np.float64)
    for i in range(n):
        Rh = c[i] * I3 + s[i] * K + (1 - c[i]) * A
        Sm = sat[i] * I3 + (1 - sat[i]) * L
        Mc[i] = bright[i] * (Sm @ Rh)
    qu = cutout_u.astype(np.float64)
    capply = qu[:, 0] < 0.5
    return (Ms, Mc, gamma, capply, qu[:, 1], qu[:, 2],
            0.3 + 0.2 * qu[:, 3], 0.3 + 0.2 * qu[:, 4])


def tile_bbox(M, ty, txi):
    ys = np.array([ty * TILE_H, ty * TILE_H + TILE_H - 1], np.float64)
    xs = np.array([txi * TILE_W, txi * TILE_W + TILE_W - 1], np.float64)
    Xg, Yg = np.meshgrid(xs, ys)
    xn = Xg * SCL - 1
    yn = Yg * SCL - 1
    nu = M[0, 0] * xn + M[0, 1] * yn + M[0, 2]
    nv = M[1, 0] * xn + M[1, 1] * yn + M[1, 2]
    de = M[2, 0] * xn + M[2, 1] * yn + M[2, 2]
    px = np.clip((nu / de + 1) * 0.5 * 511, 0, 511)
    py = np.clip((nv / de + 1) * 0.5 * 511, 0, 511)
    return (int(np.floor(py.min())), int(np.floor(py.max())),
            int(np.floor(px.min())), int(np.floor(px.max())))


def global_band_shape(Ms):
    RB = CB = 0
    for bi in range(Ms.shape[0]):
        for ty in range(NTY):
            for txi in range(NTX):
                y0, y1, x0, x1 = tile_bbox(Ms[bi], ty, txi)
                RB = max(RB, y1 - y0 + 3)
                CB = max(CB, x1 - x0 + 3)
    assert (RB + 1) * CB * CH <= 2**15, f"band too big: {RB}x{CB}"
    return RB, CB


def build_tables(Ms, Mc, gamma, capply, ccx, ccy, ccw, cch, core, nround,
                 RB, CB, sim_floor):
    dmaoff = np.zeros((nround, TPR), np.int32)
    coef = np.zeros((nround, P, NCOEF), np.float64)
    pidx = np.arange(P)
    # A floor fold: on HW cast rounds-to-nearest -> px-0.5; sim cast truncs.
    FA = 255.5 if sim_floor else 255.0
    for r in range(nround):
        imgl, strip = r // 16, r % 16
        bi = core * IPC + imgl
        M = Ms[bi]
        TY0 = 32 * strip
        t0 = SCL * TY0 - 1.0
        cf = coef[r]
        # slopes (same for every partition; image constant per round)
        cf[:, A_SU] = (255.5 * M[0, 1] + FA * M[2, 1]) * SCL
        cf[:, A_SV] = (255.5 * M[1, 1] + FA * M[2, 1]) * SCL
        cf[:, A_SD] = M[2, 1] * SCL
        cf[:, B_SU] = 2 * (255.5 * M[0, 1] + 255.5 * M[2, 1]) * SCL
        cf[:, B_SV] = 2 * (255.5 * M[1, 1] + 255.5 * M[2, 1]) * SCL
        cf[:, B_SD] = 2 * M[2, 1] * SCL
        # cutout box (image coords)
        if capply[bi]:
            lx = 511.0 * (ccx[bi] - ccw[bi] * 0.5)
            hx = 511.0 * (ccx[bi] + ccw[bi] * 0.5)
            ly = 511.0 * (ccy[bi] - cch[bi] * 0.5)
            hy = 511.0 * (ccy[bi] + cch[bi] * 0.5)
        # per-tile data
        Y0s = np.zeros(TPR, np.int64)
        X0s = np.zeros(TPR, np.int64)
        for t in range(TPR):
            y0, y1, x0, x1 = tile_bbox(M, strip, t)
            Y0 = max(0, min(y0 - 1, H - 1 - RB))
            X0 = max(0, min(x0 - 1, W - 1 - CB))
            Y0s[t], X0s[t] = Y0, X0
            dmaoff[r, t] = imgl * IMGEL + Y0 * ROWB + X0 * CH
        # ---- layout A (g = p//16, m = p%16, quarter q -> tile 4g+q) ----
        for q in range(4):
            tt = (pidx // 16) * 4 + q           # tile per partition
            TX0 = 16 * tt
            xn = SCL * (TX0 + (pidx % 16)) - 1.0
            nui = M[0, 0] * xn + M[0, 1] * t0 + M[0, 2]
            nvi = M[1, 0] * xn + M[1, 1] * t0 + M[1, 2]
            dei = M[2, 0] * xn + M[2, 1] * t0 + M[2, 2]
            cf[:, A_BU + q] = 255.5 * nui + FA * dei
            cf[:, A_BV + q] = 255.5 * nvi + FA * dei
            cf[:, A_BD + q] = dei
            cf[:, A_IDXB + q] = -(Y0s[tt] * (CB + 1) + X0s[tt])
        # ---- layout B (G = p//32, kk = p%32, segs (h,q)) ----
        kk = pidx % 32
        xB = kk % 16
        par = kk // 16
        ypart = SCL * (TY0 + par) - 1.0
        for h in range(2):
            for q in range(4):
                tt = ((pidx // 32) * 2 + h) * 4 + q
                TX0 = 16 * tt
                xn = SCL * (TX0 + xB) - 1.0
                nui = M[0, 0] * xn + M[0, 1] * ypart + M[0, 2]
                nvi = M[1, 0] * xn + M[1, 1] * ypart + M[1, 2]
                dei = M[2, 0] * xn + M[2, 1] * ypart + M[2, 2]
                sg = 4 * h + q
                cf[:, B_BU + sg] = 255.5 * (nui + dei)
                cf[:, B_BV + sg] = 255.5 * (nvi + dei)
                cf[:, B_BD + sg] = dei
                if capply[bi]:
                    xg = TX0 + xB
                    xin = (xg >= np.floor(lx) + 1) & (xg <= np.ceil(hx) - 1)
                    cly = np.where(xin, np.floor(ly) + 0.5 - TY0, 5000.0)
                    chy = np.where(xin, np.ceil(hy) - 0.5 - TY0, -5000.0)
                else:
                    cly, chy = 5000.0, -5000.0
                cf[:, B_CLY + sg] = cly
                cf[:, B_CHY + sg] = chy
        # ---- dense (tile = p//4) ----
        cf[:, D_G] = gamma[bi]
        cf[:, D_M:D_M + 9] = Mc[bi].reshape(-1)
    return dmaoff, coef.astype(np.float32)


def build_wsel():
    # phase (b, q): out partition 32*b + tile (tile = 4g+q) sums the 4 tap
    # partitions 16g + 4q + t; moving slice is gout[:, 1536q + 384b : +384].
    ws = np.zeros((16, P, P), np.float32)
    for bq in range(16):
        b, q = bq // 4, bq % 4
        for g in range(8):
            i = 32 * b + 4 * g + q
            for t in range(4):
                ws[bq, 16 * g + 4 * q + t, i] = 1.0
    return ws.transpose(1, 0, 2).reshape(P, 16 * P).copy()


def build_kernel(RB, CB, nround, sim_floor=False):
    nc = bacc.Bacc("TRN2", target_bir_lowering=False, debug=False,
                   enable_asserts=False, num_devices=NCORES)
    img = nc.dram_tensor("img", [IPC * IMGEL], F32, kind="ExternalInput")
    dmaoff_d = nc.dram_tensor("dmaoff", [nround, TPR], I32,
                              kind="ExternalInput")
    coef_d = nc.dram_tensor("coef", [nround, P, NCOEF], F32,
                            kind="ExternalInput")
    wsel_d = nc.dram_tensor("wsel", [P, 16 * P], F32, kind="ExternalInput")
    pconst_d = nc.dram_tensor("pconst", [P, 1], F32, kind="ExternalInput")
    out_d = nc.dram_tensor("out", [IPC * IMGEL], F32, kind="ExternalOutput")
    CBL = CB + 1                          # band row width: +1 px so the
    CB3 = CBL * CH                        # x+1 shift stays within the row
    BSZ = (RB + 1) * CB3                  # elems per band copy (p0)
    GSZ = RB * CB3                        # elems the gather can address
    AX = mybir.AluOpType
    AF = mybir.ActivationFunctionType
    FLOFF = 0.0 if sim_floor else 0.5
    NREG = 8                              # offset registers per engine

    with tile.TileContext(nc) as tc:
        with tc.tile_pool(name="cpool", bufs=1) as cpool, \
             tc.tile_pool(name="wpool", bufs=2) as wpool, \
             tc.tile_pool(name="ppool", bufs=2, space="PSUM") as ppool:
            # ---- persistent constants ----
            ii = cpool.tile([P, P], I32, name="ii")
            yA = cpool.tile([P, P], F32, name="yA")
            uB = cpool.tile([P, P], F32, name="uB")
            yinB = cpool.tile([P, P], F32, name="yinB")
            parc = cpool.tile([P, 1], F32, name="parc")
            wsel = cpool.tile([P, 16 * P], F32, name="wsel")
            nc.sync.dma_start(wsel[:], wsel_d[:, :])
            # yA: value = k % 32 over cols k
            nc.gpsimd.iota(ii[:], [[0, 4], [1, 32]], channel_multiplier=0)
            nc.vector.tensor_copy(yA[:], ii[:])
            # uB: cols (h:2, q:4, u16:16) -> u16
            nc.gpsimd.iota(ii[:], [[0, 2], [0, 4], [1, 16]],
                           channel_multiplier=0)
            nc.vector.tensor_copy(uB[:], ii[:])
            # par = (p % 32) >= 16 per partition (host-provided)
            nc.sync.dma_start(parc[:], pconst_d[:, :])
            # yinB = 2*u16 + par
            nc.vector.tensor_scalar(yinB[:], uB[:], 2.0, None, AX.mult)
            nc.vector.tensor_scalar(yinB[:], yinB[:], parc[:, 0:1], None,
                                    AX.add)

            # ---- persistent w_in (unused cols stay zero/stale-finite) ----
            w_in = cpool.tile([P, POS], F32, name="w_in")
            nc.gpsimd.memset(w_in[:], 0.0)

            # ---- manually double-buffered band tiles ----
            bands = [cpool.tile([P, BSZ], F32, name=f"band{i}")
                     for i in range(2)]
            # p1/p2/p3 tap copies fall a few elems short of the full band
            # extent; zero both buffers once so those cells stay finite.
            for bd in bands:
                nc.gpsimd.memset(bd[:], 0.0)

            # offset registers: NREG per HWDGE engine so reg_loads pipeline
            regs = {"sync": [], "scalar": []}
            for nm, et in (("sync", mybir.EngineType.SP),
                           ("scalar", mybir.EngineType.Activation)):
                for i in range(NREG):
                    regs[nm].append(nc.alloc_register(et, f"boff_{nm}{i}"))

            bos = [cpool.tile([1, TPR], I32, name=f"bo{i}")
                   for i in range(2)]
            cfs = [cpool.tile([P, NCOEF], F32, name=f"cf{i}")
                   for i in range(3)]

            def load_cf(r):
                nc.scalar.dma_start(
                    cfs[r % 3][:],
                    coef_d[r:r + 1, :, :].rearrange("o p c -> (o p) c"))

            def issue_loads(r):
                bd, bo = bands[r % 2], bos[r % 2]
                i_bo = nc.sync.dma_start(bo[:], dmaoff_d[r:r + 1, :])
                # batch reg_loads, then the dma_starts that consume them
                for half, eng in ((0, "sync"), (1, "scalar")):
                    e = getattr(nc, eng)
                    for blk in range(16 // NREG):
                        for i in range(NREG):
                            t = 16 * half + blk * NREG + i
                            rl = nc.reg_load(regs[eng][i], bo[0:1, t:t + 1])
                            add_dep_helper(rl.ins, i_bo.ins, sync=True,
                                           reason="boff")
                        for i in range(NREG):
                            t = 16 * half + blk * NREG + i
                            src = bass.AP(img, regs[eng][i],
                                          [[ROWB, RB + 1], [1, CB3]])
                            p0 = 4 * t
                            e.dma_start(bd[p0:p0 + 1, 0:BSZ], src)

            def issue_reps(r):
                # tap-shift replication: p0 -> p1 (x+1), p2 (y+1), p3 (both)
                bd = bands[r % 2]
                if sim_floor:
                    # interp can't handle strided-partition SBUF copies
                    for t in range(TPR):
                        p0 = 4 * t
                        nc.sync.dma_start(bd[p0 + 1:p0 + 2, 0:GSZ],
                                          bd[p0:p0 + 1, 3:GSZ + 3])
                        nc.scalar.dma_start(bd[p0 + 2:p0 + 3, 0:GSZ],
                                            bd[p0:p0 + 1, CB3:CB3 + GSZ])
                        nc.sync.dma_start(bd[p0 + 3:p0 + 4, 0:GSZ - 3],
                                          bd[p0:p0 + 1, CB3 + 3:CB3 + GSZ])
                else:
                    # SWDGE: one instruction's descriptors spray across all
                    # 16 SDMA engines (HWDGE statics pin to one queue each)
                    nc.gpsimd.dma_start(bd[1:P:4, 0:GSZ],
                                        bd[0:P:4, 3:GSZ + 3])
                    nc.gpsimd.dma_start(bd[2:P:4, 0:GSZ],
                                        bd[0:P:4, CB3:CB3 + GSZ])
                    nc.gpsimd.dma_start(bd[3:P:4, 0:GSZ - 3],
                                        bd[0:P:4, CB3 + 3:CB3 + GSZ])

            def emit_layouts(r):
                """DVE block computing round r's gather indices and tap
                weights; in steady state it runs during round r-1."""
                cf = cfs[r % 3]

                def sc(i):
                    return cf[:, i:i + 1]

                def bcA(i):
                    b = cf[:, i:i + 4]
                    return bass.AP(b.tensor, b.offset,
                                   [b.ap[0], [1, 4], [0, 32]])

                def bcB(i):
                    b = cf[:, i:i + 8]
                    return bass.AP(b.tensor, b.offset,
                                   [b.ap[0], [1, 8], [0, 16]])

                # ---- layout A: gather indices ----
                de = wpool.tile([P, P], F32, tag="de")
                t1 = wpool.tile([P, P], F32, tag="t1")
                t2 = wpool.tile([P, P], F32, tag="t2")
                x0f = wpool.tile([P, P], F32, tag="x0f")
                y0f = wpool.tile([P, P], F32, tag="y0f")
                s_i = wpool.tile([P, P], I32, tag="s_i")
                idx16 = wpool.tile([P, P], I16, tag="idx16")
                nc.vector.scalar_tensor_tensor(
                    de[:], yA[:], sc(A_SD), bcA(A_BD), AX.mult, AX.add)
                nc.vector.reciprocal(de[:], de[:])
                for dst, scol, bcol in ((x0f, A_SU, A_BU), (y0f, A_SV, A_BV)):
                    nc.vector.scalar_tensor_tensor(
                        t1[:], yA[:], sc(scol), bcA(bcol), AX.mult, AX.add)
                    nc.vector.tensor_tensor(t1[:], t1[:], de[:], AX.mult)
                    nc.vector.tensor_copy(s_i[:], t1[:])
                    nc.vector.tensor_copy(dst[:], s_i[:])
                    nc.vector.tensor_scalar(dst[:], dst[:], 0.0, 510.0,
                                            AX.max, AX.min)
                # idx = y0f*CBL + x0f + bias -> i16
                nc.vector.tensor_scalar(y0f[:], y0f[:], float(CBL), None,
                                        AX.mult)
                nc.vector.tensor_tensor(y0f[:], y0f[:], x0f[:], AX.add)
                nc.vector.tensor_tensor(y0f[:], y0f[:], bcA(A_IDXB), AX.add)
                nc.vector.tensor_copy(s_i[:], y0f[:])
                nc.vector.tensor_copy(idx16[:], s_i[:])

                # ---- layout B: tap weights ----
                pxu = wpool.tile([P, P], F32, tag="pxu")
                pyu = wpool.tile([P, P], F32, tag="pyu")
                pxc = wpool.tile([P, P], F32, tag="pxc")
                pyc = wpool.tile([P, P], F32, tag="pyc")
                vv = wpool.tile([P, P], F32, tag="vv")
                nc.vector.scalar_tensor_tensor(
                    de[:], uB[:], sc(B_SD), bcB(B_BD), AX.mult, AX.add)
                nc.vector.reciprocal(de[:], de[:])
                for dst, dstc, scol, bcol in ((pxu, pxc, B_SU, B_BU),
                                              (pyu, pyc, B_SV, B_BV)):
                    nc.vector.scalar_tensor_tensor(
                        t1[:], uB[:], sc(scol), bcB(bcol), AX.mult, AX.add)
                    nc.vector.tensor_tensor(dst[:], t1[:], de[:], AX.mult)
                    nc.vector.tensor_scalar(dstc[:], dst[:], 0.0, 511.0,
                                            AX.max, AX.min)
                # valid
                nc.vector.tensor_tensor(pxu[:], pxu[:], pxc[:], AX.is_equal)
                nc.vector.tensor_tensor(pyu[:], pyu[:], pyc[:], AX.is_equal)
                nc.vector.tensor_tensor(vv[:], pxu[:], pyu[:], AX.mult)
                # cutout: ym in (CLY, CHY) -> zero weight
                nc.vector.tensor_tensor(t1[:], yinB[:], bcB(B_CLY), AX.is_ge)
                nc.vector.tensor_tensor(t2[:], yinB[:], bcB(B_CHY), AX.is_le)
                nc.vector.tensor_tensor(t1[:], t1[:], t2[:], AX.mult)
                nc.vector.tensor_scalar(t1[:], t1[:], -1.0, 1.0,
                                        AX.mult, AX.add)
                nc.vector.tensor_tensor(vv[:], vv[:], t1[:], AX.mult)
                # floors + fracs
                for pc, fl in ((pxc, x0f), (pyc, y0f)):
                    nc.vector.tensor_scalar(t1[:], pc[:], 1.0, -FLOFF,
                                            AX.mult, AX.add)
                    nc.vector.tensor_copy(s_i[:], t1[:])
                    nc.vector.tensor_copy(fl[:], s_i[:])
                    nc.vector.tensor_scalar(fl[:], fl[:], 0.0, 510.0,
                                            AX.max, AX.min)
                    nc.vector.tensor_tensor(pc[:], pc[:], fl[:], AX.subtract)
                # pxc=wx, pyc=wy now. basis: t1=A1=V*wx, t2=A0=V-A1, vv ->
                nc.vector.tensor_tensor(t1[:], vv[:], pxc[:], AX.mult)
                nc.vector.tensor_tensor(t2[:], vv[:], t1[:], AX.subtract)
                # B0 = 1-wy -> pxu (reuse)
                nc.vector.tensor_scalar(pxu[:], pyc[:], -1.0, 1.0,
                                        AX.mult, AX.add)
                # tap writes into w_in (cols 516q + 32u16 + 16h + t)
                for hh in range(2):
                    for t in range(4):
                        adx = t1 if (t & 1) else t2
                        bdy = pyc if (t & 2) else pxu
                        sb = adx[:, 64 * hh:64 * hh + 64]
                        sv = bass.AP(sb.tensor, sb.offset,
                                     [sb.ap[0], [16, 4], [1, 16]])
                        db = w_in[:, 16 * hh + t:]
                        dv = bass.AP(db.tensor, db.offset,
                                     [db.ap[0], [516, 4], [32, 16]])
                        bb = bdy[:, 64 * hh:64 * hh + 64]
                        bv = bass.AP(bb.tensor, bb.offset,
                                     [bb.ap[0], [16, 4], [1, 16]])
                        nc.vector.tensor_tensor(dv, sv, bv, AX.mult)
                # transpose to gather alignment
                w_out = wpool.tile([P, POS], F32, tag="w_out")
                nc.vector.transpose(w_out[:], w_in[:])
                return idx16, w_out

            # ---- prologue: prime round 0 ----
            load_cf(0)
            issue_loads(0)
            issue_reps(0)
            if nround > 1:
                load_cf(1)
            idx_cur, w_cur = emit_layouts(0)

            for r in range(nround):
                band = bands[r % 2]
                cf = cfs[r % 3]

                def scr(i, cf=cf):
                    return cf[:, i:i + 1]

                if r + 2 < nround:
                    load_cf(r + 2)

                # ---- gather (indices computed during round r-1) ----
                gout = wpool.tile([P, POS * CH], F32, tag="gout")
                i_gth = nc.gpsimd.ap_gather(
                    gout[:].rearrange("p (n d) -> p n d", d=CH),
                    band[:, 0:GSZ].rearrange("p (n d) -> p n d", d=CH),
                    idx_cur[:], channels=P, num_elems=RB * CBL, d=CH,
                    num_idxs=POS)

                # next round's band + DVE layouts overlap this round
                if r + 1 < nround:
                    issue_loads(r + 1)
                    idx_nxt, w_nxt = emit_layouts(r + 1)
                else:
                    idx_nxt = w_nxt = None

                # ---- apply weights (computed during round r-1) ----
                wb = bass.AP(w_cur.tensor, w_cur[:].offset,
                             [w_cur[:].ap[0], [1, POS], [0, CH]])
                i_wm = nc.gpsimd.tensor_tensor(
                    gout[:].rearrange("p (n d) -> p n d", d=CH),
                    gout[:].rearrange("p (n d) -> p n d", d=CH),
                    wb, AX.mult)
                add_dep_helper(i_wm.ins, i_gth.ins, sync=True, reason="graw")

                if r + 1 < nround:
                    issue_reps(r + 1)

                # ---- combine via matmul: psum IS the dense layout ----
                pt = ppool.tile([P, 3 * S // 4], F32, tag="psum")
                for bq in range(16):
                    b, q = bq // 4, bq % 4
                    nc.tensor.matmul(
                        pt[:, :],
                        wsel[:, P * bq:P * (bq + 1)],
                        gout[:, 1536 * q + 384 * b:
                             1536 * q + 384 * b + 384],
                        start=(bq == 0), stop=(bq == 15))

                # ---- post: clip, gamma, color, clip ----
                d_t = wpool.tile([P, 3 * S // 4], F32, tag="d_t")
                nc.vector.tensor_scalar(pt[:], pt[:], 1e-30, 1.0,
                                        AX.max, AX.min)
                nc.scalar.activation(d_t[:], pt[:], AF.Ln)
                nc.scalar.activation(d_t[:], d_t[:], AF.Exp,
                                     scale=cf[:, D_G:D_G + 1])
                d_c = wpool.tile([P, 3 * S // 4], F32, tag="d_c")
                dv3 = d_t[:].rearrange("p (n c) -> p n c", c=CH)
                tv3 = d_c[:].rearrange("p (n c) -> p n c", c=CH)
                for c3 in range(3):
                    nc.vector.tensor_scalar(
                        tv3[:, :, c3:c3 + 1], dv3[:, :, 0:1],
                        scr(D_M + 3 * c3), None, AX.mult)
                    nc.vector.scalar_tensor_tensor(
                        tv3[:, :, c3:c3 + 1], dv3[:, :, 1:2],
                        scr(D_M + 3 * c3 + 1), tv3[:, :, c3:c3 + 1],
                        AX.mult, AX.add)
                    nc.vector.scalar_tensor_tensor(
                        tv3[:, :, c3:c3 + 1], dv3[:, :, 2:3],
                        scr(D_M + 3 * c3 + 2), tv3[:, :, c3:c3 + 1],
                        AX.mult, AX.add)
                nc.vector.tensor_scalar(d_c[:], d_c[:], 0.0, 1.0,
                                        AX.max, AX.min)

                # ---- output stores ----
                imgl, strip = r // 16, r % 16
                for r2 in range(8):
                    off = imgl * IMGEL + (32 * strip + r2) * ROWB
                    dst = bass.AP(out_d, off,
                                  [[8 * ROWB, 4], [48, 32], [1, 48]])
                    e = nc.sync if r2 % 2 == 0 else nc.scalar
                    e.dma_start(dst, d_c[:, 48 * r2:48 * r2 + 48])

                idx_cur, w_cur = idx_nxt, w_nxt
    nc.compile()
    return nc


_CACHE = {}


def kernel(images, geom_u, color_u, cutout_u):
    _install_profhook()
    images = np.asarray(images, dtype=np.float32)
    geom_u = np.asarray(geom_u, dtype=np.float32)
    color_u = np.asarray(color_u, dtype=np.float32)
    cutout_u = np.asarray(cutout_u, dtype=np.float32)
    Ms, Mc, gamma, capply, ccx, ccy, ccw, cch = host_params(
        geom_u, color_u, cutout_u)
    RB, CB = global_band_shape(Ms)
    nround = int(os.environ.get('KROUNDS', NROUND_FULL))
    sim_floor = os.environ.get("KSIM", "0") == "1"
    key = (RB, CB, nround, sim_floor)
    if key not in _CACHE:
        _CACHE[key] = build_kernel(RB, CB, nround, sim_floor=sim_floor)
    nc = _CACHE[key]
    wsel = build_wsel()
    in_maps = []
    for core in range(NCORES):
        dmaoff, coef = build_tables(Ms, Mc, gamma, capply, ccx, ccy, ccw,
                                    cch, core, nround, RB, CB, sim_floor)
        shard = images[core * IPC:(core + 1) * IPC].reshape(-1)
        pconst = (((np.arange(P) % 32) >= 16)
                  .astype(np.float32).reshape(P, 1))
        in_maps.append({"img": np.ascontiguousarray(shard),
                        "dmaoff": dmaoff, "coef": coef, "wsel": wsel,
                        "pconst": pconst})
    if sim_floor:
        from concourse.bass_interp import CoreSim
        sim = CoreSim(nc, trace=False)
        for k, v in in_maps[0].items():
            sim.tensor(k)[:] = v
        sim.simulate()
        out = np.zeros((B, H, W, CH), np.float32)
        out[0:IPC] = np.array(sim.tensor("out")).reshape(IPC, H, W, CH)
        return out
    trace = os.environ.get("KTRACE", "0") == "1"
    res = run_bass_kernel_spmd(nc, in_maps, core_ids=list(range(NCORES)),
                               trace=trace, trace_cores=[0] if trace else None)
    if trace and res.exec_time_ns:
        print(f"HW exec time: {res.exec_time_ns} ns")
    out = np.zeros((B, H, W, CH), np.float32)
    for core in range(NCORES):
        out[core * IPC:(core + 1) * IPC] = (
            res.results[core]["out"].reshape(IPC, H, W, CH))
    return out
